# revision 1
# baseline (speedup 1.0000x reference)
"""CGRUCell Trainium2 kernel — hybrid: data-parallel x4 over batch,
tensor-parallel x2 over units, on 8 NeuronCores.

Core c: batch group g=c//2 (256 rows), unit parity p=c%2 (units
[p*1024:(p+1)*1024], i.e. its real+imag output columns). Weights are
split by parity (100.7 MB/core instead of 201 MB replicated). The only
cross-core data is r*h for the candidate gate: a pairwise AllGather,
overlapped with gate-z matmuls.

Gate r is computed output-transposed (weights stationary, activations
moving) so r and r*h are produced directly in K-major layout — no
on-chip transposes. Gates z and h use the batch-stationary orientation
(N=512 moving) for better weight-load amortization.

Matmuls run in float32r (TF32-like full-rate fp32 mode, rel err ~1e-4).
Gate order: r -> z -> h so the r*h exchange hides under z's compute.
"""

import sys

for _p in ("/opt/trn_rl_repo", "/root/.axon_site/_ro/trn_rl_repo"):
    if _p not in sys.path:
        sys.path.append(_p)

import numpy as np

import concourse.bass as bass
import concourse.mybir as mybir
import concourse.tile as tile
from concourse import bacc
from concourse.bass_utils import run_bass_kernel_spmd

P = 128
U = 2048           # UNITS
B = 1024
N_CORES = 8
GROUPS = 4
BC = B // GROUPS   # 256 batch rows per core
MSUB = BC // P     # 2 m-subtiles
UC = U // 2        # 1024 unit columns per core (per half)
KCH = U // P       # 16 k-chunks per complex half
F32 = mybir.dt.float32
MM_DT = mybir.dt.float32r
NBLK = UC // 512   # 2 col-blocks of 512 per half
BLK = 512

_CACHE = {}


def _build_nc(repeat=1):
    nc = bacc.Bacc(None, target_bir_lowering=False)

    # K-major activations (full contraction dims, batch 256 of this group)
    x1 = nc.dram_tensor("x1", [P, KCH, BC], MM_DT, kind="ExternalInput")
    x2 = nc.dram_tensor("x2", [P, KCH, BC], MM_DT, kind="ExternalInput")
    h1 = nc.dram_tensor("h1", [P, KCH, BC], MM_DT, kind="ExternalInput")
    h2 = nc.dram_tensor("h2", [P, KCH, BC], MM_DT, kind="ExternalInput")
    # batch-major h_tm1, own columns only: [256, 2048]
    hbm = nc.dram_tensor("hbm", [BC, 2 * UC], F32, kind="ExternalInput")
    # K-major h_tm1, own columns: [p, o(8 re + 8 im), b]
    hTo = nc.dram_tensor("hTo", [P, KCH, BC], MM_DT, kind="ExternalInput")
    # weights, parity-sliced on host: [2048, 3*1024] (gate z|r|h blocks)
    RK = nc.dram_tensor("RK", [U, 3 * UC], MM_DT, kind="ExternalInput")
    IK = nc.dram_tensor("IK", [U, 3 * UC], MM_DT, kind="ExternalInput")
    RR = nc.dram_tensor("RR", [U, 3 * UC], MM_DT, kind="ExternalInput")
    IR = nc.dram_tensor("IR", [U, 3 * UC], MM_DT, kind="ExternalInput")
    # bias for z/h, own columns, block order [gate(3), half(2), 1024]
    # (gate r entries unused, kept for layout simplicity)
    bias = nc.dram_tensor("bias", [3 * 2 * UC], F32, kind="ExternalInput")
    # gate-r bias, column-major pre-scaled: b' = 0.2*b + 0.5, [128, 16]
    biasr = nc.dram_tensor("biasr", [P, KCH], F32, kind="ExternalInput")
    out = nc.dram_tensor("out", [BC, 2 * UC], F32, kind="ExternalOutput")

    hbm_r = hbm.rearrange("(m p) c -> p m c", p=P)
    out_r = out.rearrange("(m p) c -> p m c", p=P)

    with tile.TileContext(nc) as tc:
        with (
            tc.tile_pool(name="acts", bufs=1) as acts,
            tc.tile_pool(name="wpool", bufs=2) as wpool,
            tc.tile_pool(name="psum", bufs=1, space="PSUM") as psum,
            tc.tile_pool(name="small", bufs=2) as small,
            tc.tile_pool(name="neg", bufs=2) as negp,
            tc.tile_pool(name="bigs", bufs=1) as bigs,
            tc.tile_pool(name="dram", bufs=1, space="DRAM") as dram,
        ):
            x1s = acts.tile([P, KCH, BC], MM_DT, tag="x1s", name="x1s")
            x2s = acts.tile([P, KCH, BC], MM_DT, tag="x2s", name="x2s")
            h1s = acts.tile([P, KCH, BC], MM_DT, tag="h1s", name="h1s")
            h2s = acts.tile([P, KCH, BC], MM_DT, tag="h2s", name="h2s")
            hTos = acts.tile([P, KCH, BC], MM_DT, tag="hTos", name="hTos")
            rh2s = acts.tile([P, KCH, BC], MM_DT, tag="rh2s", name="rh2s")
            # chunked act loads: the first k-slices land in ~2us so the
            # gate-r matmuls start without waiting for the full 10 MB
            for o in range(0, KCH, 4):
                sl = slice(o, o + 4)
                nc.sync.dma_start(x1s[:, sl, :], x1[:, sl, :])
                nc.sync.dma_start(x2s[:, sl, :], x2[:, sl, :])
                nc.sync.dma_start(h1s[:, sl, :], h1[:, sl, :])
                nc.sync.dma_start(h2s[:, sl, :], h2[:, sl, :])
                nc.sync.dma_start(hTos[:, sl, :], hTo[:, sl, :])

            hbmt = bigs.tile([P, MSUB, 2 * UC], F32, tag="hbmt", name="hbmt")
            nc.sync.dma_start(hbmt[:], hbm_r)

            z_sb = bigs.tile([P, MSUB, 2 * UC], F32, tag="z_sb", name="z_sb")
            hh_sb = bigs.tile([P, MSUB, 2 * UC], F32, tag="hh_sb", name="hh_sb")

            brcol = small.tile([P, KCH], F32, tag="brcol", name="brcol", bufs=1)
            nc.sync.dma_start(brcol[:], biasr[:])

            def wtile(rep, g, tname, k, bb, src, width=BLK):
                t = wpool.tile([P, BLK], MM_DT, tag=tname,
                               name=f"{tname}_{rep}_{g}_{k}_{bb}")
                ksl = slice(k * P, (k + 1) * P)
                csl = slice(g * UC + bb * width, g * UC + (bb + 1) * width)
                nc.sync.dma_start(t[:], src[ksl, csl])
                return t

            def gate_phase_a(rep, g, b1, b2, evict):
                """Batch-stationary gates (z, h): 8 psum blocks
                [m, half, bb] of [128 batch, 512 cols]."""
                ps = {}
                for m in range(MSUB):
                    for half in range(2):
                        for bb in range(NBLK):
                            ps[(m, half, bb)] = psum.tile(
                                [P, BLK], F32,
                                tag=f"ps{(m * 2 + half) * NBLK + bb}",
                                name=f"ps_{rep}_{g}_{m}_{half}_{bb}",
                            )
                for k in range(KCH):
                    wts = {
                        n: [wtile(rep, g, f"{n}{bb}", k, bb, src)
                            for bb in range(NBLK)]
                        for n, src in (("rk", RK), ("ik", IK),
                                       ("rr", RR), ("ir", IR))
                    }
                    na1 = negp.tile([P, BC], MM_DT, tag="na1",
                                    name=f"na1_{rep}_{g}_{k}")
                    nb1 = negp.tile([P, BC], MM_DT, tag="nb1",
                                    name=f"nb1_{rep}_{g}_{k}")
                    nc.vector.tensor_scalar(
                        na1[:], x1s[:, k, :], -1.0, None, mybir.AluOpType.mult
                    )
                    nc.vector.tensor_scalar(
                        nb1[:], b1[:, k, :], -1.0, None, mybir.AluOpType.mult
                    )

                    first = k == 0
                    last = k == KCH - 1
                    streams = [
                        (x1s, "rk", 0, first, False),
                        (x2s, "rk", 1, first, False),
                        (x2s, "ik", 0, False, False),
                        (na1, "ik", 1, False, False),
                        (b1, "rr", 0, False, False),
                        (b2, "rr", 1, False, False),
                        (b2, "ir", 0, False, last),
                        (nb1, "ir", 1, False, last),
                    ]
                    for stat, wn, half, st, sp in streams:
                        for m in range(MSUB):
                            if stat is na1 or stat is nb1:
                                lhsT = stat[:, m * P : (m + 1) * P]
                            else:
                                lhsT = stat[:, k, m * P : (m + 1) * P]
                            for bb in range(NBLK):
                                nc.tensor.matmul(
                                    ps[(m, half, bb)],
                                    lhsT,
                                    wts[wn][bb][:],
                                    start=st,
                                    stop=sp,
                                )

                for half in range(2):
                    for bb in range(NBLK):
                        j = half * NBLK + bb
                        bt = small.tile([P, BLK], F32, tag="bt",
                                        name=f"bt_{rep}_{g}_{j}")
                        nc.sync.dma_start(
                            bt[:],
                            bias[None, g * 2 * UC + j * BLK : g * 2 * UC
                                 + (j + 1) * BLK].to_broadcast((P, BLK)),
                        )
                        for m in range(MSUB):
                            oc = slice(half * UC + bb * BLK,
                                       half * UC + (bb + 1) * BLK)
                            evict(ps[(m, half, bb)], bt, m, oc)

            def evict_hs(dest):
                def _e(pst, bt, m, oc):
                    d = dest[:, m, oc]
                    nc.vector.tensor_add(d, pst[:], bt[:])
                    nc.vector.tensor_scalar(
                        d, d, 0.2, 0.5,
                        mybir.AluOpType.mult, mybir.AluOpType.add,
                    )
                    nc.vector.tensor_scalar(
                        d, d, 1.0, 0.0,
                        mybir.AluOpType.min, mybir.AluOpType.max,
                    )
                return _e

            def evict_tanh(dest):
                def _e(pst, bt, m, oc):
                    tmp = small.tile([P, BLK], F32, tag="ttmp", name="ttmp")
                    nc.vector.tensor_add(tmp[:], pst[:], bt[:])
                    nc.scalar.activation(
                        dest[:, m, oc], tmp[:], mybir.ActivationFunctionType.Tanh
                    )
                return _e

            def gate_r_transposed(rep, rhTl):
                """Gate r, output-transposed: psum [128 cols, 256 batch].
                grp 0 covers weight cols 0:512 (real ccs 0-3 + imag ccs 0-3),
                grp 1 covers cols 512:1024. Writes rhT = hs(pre_r)*hT
                directly into rhTl[:, ccg, :]."""
                g = 1
                for grp in range(2):
                    ps = [
                        psum.tile([P, BC], F32, tag=f"ps{i}",
                                  name=f"psr_{rep}_{grp}_{i}")
                        for i in range(8)
                    ]
                    # ps[0..3] real ccs, ps[4..7] imag ccs
                    for k in range(KCH):
                        wts = {
                            n: wtile(rep, g, f"{n}{grp}", k, grp, src)
                            for n, src in (("rk", RK), ("ik", IK),
                                           ("rr", RR), ("ir", IR))
                        }
                        na1 = negp.tile([P, BC], MM_DT, tag="na1",
                                        name=f"na1r_{rep}_{grp}_{k}")
                        nb1 = negp.tile([P, BC], MM_DT, tag="nb1",
                                        name=f"nb1r_{rep}_{grp}_{k}")
                        nc.vector.tensor_scalar(
                            na1[:], x1s[:, k, :], -1.0, None,
                            mybir.AluOpType.mult,
                        )
                        nc.vector.tensor_scalar(
                            nb1[:], h1s[:, k, :], -1.0, None,
                            mybir.AluOpType.mult,
                        )
                        first = k == 0
                        last = k == KCH - 1
                        # (weight, moving, psum base, start, stop)
                        streams = [
                            ("rk", x1s, 0, first, False),
                            ("rk", x2s, 4, first, False),
                            ("ik", x2s, 0, False, False),
                            ("ik", na1, 4, False, False),
                            ("rr", h1s, 0, False, False),
                            ("rr", h2s, 4, False, False),
                            ("ir", h2s, 0, False, last),
                            ("ir", nb1, 4, False, last),
                        ]
                        for wn, mov, base, st, sp in streams:
                            if mov is na1 or mov is nb1:
                                rhs = mov[:]
                            else:
                                rhs = mov[:, k, :]
                            for cc in range(4):
                                nc.tensor.matmul(
                                    ps[base + cc],
                                    wts[wn][:, cc * P : (cc + 1) * P],
                                    rhs,
                                    start=st,
                                    stop=sp,
                                )
                    # evict: rhT[ccg] = clip(0.2*psum + b') * hT_own[ccg]
                    for i in range(8):
                        half = i // 4          # 0 real, 1 imag
                        ccg = half * 8 + grp * 4 + (i % 4)
                        d = rhTl[:, ccg, :]
                        nc.vector.tensor_scalar(
                            d, ps[i][:], 0.2, brcol[:, ccg : ccg + 1],
                            mybir.AluOpType.mult, mybir.AluOpType.add,
                        )
                        nc.vector.tensor_scalar(
                            d, d, 1.0, 0.0,
                            mybir.AluOpType.min, mybir.AluOpType.max,
                        )
                        nc.vector.tensor_mul(d, d, hTos[:, ccg, :])

            for rep in range(repeat):
                # --- gate r first (g=1), output-transposed ---
                rhTl = acts.tile([P, KCH, BC], MM_DT, tag="rh1s",
                                 name=f"rhTl_{rep}")
                gate_r_transposed(rep, rhTl)

                # pairwise AllGather of rhT
                inb = dram.tile([P, KCH, BC], MM_DT, tag="inb",
                                name=f"inb_{rep}")
                outb = dram.tile([2, P, KCH, BC], MM_DT, tag="outb",
                                 name=f"outb_{rep}")
                nc.sync.dma_start(inb[:], rhTl[:])
                nc.gpsimd.collective_compute(
                    "AllGather",
                    mybir.AluOpType.bypass,
                    replica_groups=[[0, 1], [2, 3], [4, 5], [6, 7]],
                    ins=[inb[:].opt()],
                    outs=[outb[:].opt()],
                )
                rh1s = acts.tile([P, KCH, BC], MM_DT, tag="rh1s",
                                 name=f"rh1s_{rep}")
                # real rows: parity0 units 0:1024 -> o 0..7, parity1 -> 8..15
                nc.sync.dma_start(rh1s[:, 0:8, :], outb[0, :, 0:8, :])
                nc.sync.dma_start(rh1s[:, 8:16, :], outb[1, :, 0:8, :])
                nc.sync.dma_start(rh2s[:, 0:8, :], outb[0, :, 8:16, :])
                nc.sync.dma_start(rh2s[:, 8:16, :], outb[1, :, 8:16, :])

                # --- gate z (g=0), overlaps with the collective ---
                gate_phase_a(rep, 0, h1s, h2s, evict_hs(z_sb))

                # --- gate h (g=2) ---
                gate_phase_a(rep, 2, rh1s, rh2s, evict_tanh(hh_sb))

                # h_new = hh + z*(h - hh), in place into hbmt;
                # per m-subtile so DVE of one overlaps the out-DMA of the other
                for m in range(MSUB):
                    nc.vector.tensor_sub(
                        hbmt[:, m, :], hbmt[:, m, :], hh_sb[:, m, :]
                    )
                    nc.vector.tensor_mul(
                        hbmt[:, m, :], z_sb[:, m, :], hbmt[:, m, :]
                    )
                    nc.vector.tensor_add(
                        hbmt[:, m, :], hh_sb[:, m, :], hbmt[:, m, :]
                    )
                    nc.sync.dma_start(out_r[:, m, :], hbmt[:, m, :])

    nc.compile()
    return nc


def _pack_kmajor(a):
    # (BC, 2048) -> (128, 16, BC) with [p, o, b] = a[b, o*128+p]
    bc = a.shape[0]
    return np.ascontiguousarray(a.T.reshape(KCH, P, bc).transpose(1, 0, 2))


def make_in_maps(
    inputs, h_tm1, real_kernel, imaginary_kernel,
    real_recurrent_kernel, imaginary_recurrent_kernel, real_bias,
    imaginary_bias,
):
    inputs = np.ascontiguousarray(inputs, dtype=np.float32)
    h_tm1 = np.ascontiguousarray(h_tm1, dtype=np.float32)
    ws = {
        "RK": np.ascontiguousarray(real_kernel, dtype=np.float32),
        "IK": np.ascontiguousarray(imaginary_kernel, dtype=np.float32),
        "RR": np.ascontiguousarray(real_recurrent_kernel, dtype=np.float32),
        "IR": np.ascontiguousarray(imaginary_recurrent_kernel, dtype=np.float32),
    }
    rb = np.asarray(real_bias, dtype=np.float32)
    ib = np.asarray(imaginary_bias, dtype=np.float32)

    wsl = {}
    bsl = {}
    brc = {}
    for p in range(2):
        cols = [slice(g * U + p * UC, g * U + (p + 1) * UC) for g in range(3)]
        wsl[p] = {
            k: np.ascontiguousarray(np.concatenate([v[:, c] for c in cols], axis=1))
            for k, v in ws.items()
        }
        bsl[p] = np.concatenate([np.concatenate([rb[c], ib[c]]) for c in cols])
        # gate-r column-major bias, pre-scaled: [128, 16], [pp, ccg]
        br = np.concatenate([rb[cols[1]], ib[cols[1]]])  # (2048,) re|im own
        brc[p] = np.ascontiguousarray(
            (0.2 * br + 0.5).reshape(KCH, P).T
        )

    in_maps = []
    for c in range(N_CORES):
        g, p = c // 2, c % 2
        rs = slice(g * BC, (g + 1) * BC)
        ocr = slice(p * UC, (p + 1) * UC)
        oci = slice(U + p * UC, U + (p + 1) * UC)
        hbm = np.ascontiguousarray(
            np.concatenate([h_tm1[rs, ocr], h_tm1[rs, oci]], axis=1)
        )
        in_maps.append(
            {
                "x1": _pack_kmajor(inputs[rs, :U]),
                "x2": _pack_kmajor(inputs[rs, U:]),
                "h1": _pack_kmajor(h_tm1[rs, :U]),
                "h2": _pack_kmajor(h_tm1[rs, U:]),
                "hbm": hbm,
                "hTo": _pack_kmajor(hbm),
                "RK": wsl[p]["RK"],
                "IK": wsl[p]["IK"],
                "RR": wsl[p]["RR"],
                "IR": wsl[p]["IR"],
                "bias": bsl[p],
                "biasr": brc[p],
            }
        )
    return in_maps


def scatter_out(results):
    h_new = np.empty((B, 2 * U), dtype=np.float32)
    for c in range(N_CORES):
        g, p = c // 2, c % 2
        rs = slice(g * BC, (g + 1) * BC)
        o = results[c]["out"]
        h_new[rs, p * UC : (p + 1) * UC] = o[:, :UC]
        h_new[rs, U + p * UC : U + (p + 1) * UC] = o[:, UC:]
    return h_new


def _build_nc_retry(repeat=1, attempts=4):
    # Tile's scheduler very occasionally reports a spurious deadlock on a
    # valid graph (ordering is not fully deterministic); retry a few times.
    last = None
    for _ in range(attempts):
        try:
            return _build_nc(repeat=repeat)
        except Exception as e:  # noqa: BLE001
            if "Deadlock" not in type(e).__name__ + str(e):
                raise
            last = e
    raise last


def kernel(
    inputs,
    h_tm1,
    real_kernel,
    imaginary_kernel,
    real_recurrent_kernel,
    imaginary_recurrent_kernel,
    real_bias,
    imaginary_bias,
):
    if "nc" not in _CACHE:
        _CACHE["nc"] = _build_nc_retry()
    nc = _CACHE["nc"]
    in_maps = make_in_maps(
        inputs, h_tm1, real_kernel, imaginary_kernel,
        real_recurrent_kernel, imaginary_recurrent_kernel, real_bias,
        imaginary_bias,
    )
    res = run_bass_kernel_spmd(nc, in_maps, core_ids=list(range(N_CORES)))
    return scatter_out(res.results)



# revision 8
# speedup vs baseline: 1.5291x; 1.5291x over previous
"""CGRUCell Trainium2 kernel — hybrid DP4 x TP2 on 8 NeuronCores, with a
3-product complex-matmul decomposition in bf16.

Core c: batch group g=c//2 (256 rows), unit parity p=c%2 (units
[p*1024:(p+1)*1024]). The complex matmul [x1 x2] @ [[Wr,-Wi],[Wi,Wr]]
is computed with 3 half-size products instead of 4 (Gauss/Karatsuba):

    A = x1 @ Wr,  B = x2 @ Wi,  C = (x2 - x1) @ (Wr + Wi)
    real = A + B,   imag = C + A - B

This cuts tensor-engine cycles by 25% vs the 4-product form (the PE
streams bf16 and fp32r at the same 1 column/cycle, so dtype alone does
not help compute; fp8-DoubleRow would, but e4m3 fails the accuracy
budget). Weights stream from HBM in bf16 (half the bytes of the fp32r
baseline); only the combination S = Wr + Wi is derived on-chip by the
vector engine, and the activation differences (x2-x1 etc.) come
pre-computed from the host, so HBM traffic stays ~65 MB/core, well
under the ~358 GB/s/core limit at the 246 us PE roofline.

z- and r-gate weight blocks are pre-scaled by 0.2 on the host so the
hard-sigmoid scale comes free; biases fold into fused
scalar_tensor_tensor / tensor_scalar evictions.

Gate r is computed output-transposed (weights stationary) so r*h is
produced directly in K-major layout for the pairwise AllGather, which
overlaps with gate z. Gates z and h are batch-stationary; per 512-col
block the three Gauss accumulators for both 128-row batch subtiles
occupy 6 PSUM banks, rotating through all 8 banks across passes.
"""

import sys

for _p in ("/opt/trn_rl_repo", "/root/.axon_site/_ro/trn_rl_repo"):
    if _p not in sys.path:
        sys.path.append(_p)

import numpy as np
import ml_dtypes

import concourse.bass as bass
import concourse.mybir as mybir
import concourse.tile as tile
from concourse import bacc
from concourse.bass_utils import run_bass_kernel_spmd

P = 128
U = 2048           # UNITS
B = 1024
N_CORES = 8
GROUPS = 4
BC = B // GROUPS   # 256 batch rows per core
MSUB = BC // P     # 2 m-subtiles
UC = U // 2        # 1024 unit columns per core (per half)
KCH = U // P       # 16 k-chunks per complex half
F32 = mybir.dt.float32
BF = mybir.dt.bfloat16
NPBF = ml_dtypes.bfloat16
NBLK = UC // 512   # 2 col-blocks of 512 per half
BLK = 512

AL = mybir.AluOpType

_CACHE = {}


def _build_nc(repeat=1):
    nc = bacc.Bacc(None, target_bir_lowering=False)

    # K-major activations: x halves, their difference, h halves + diff
    x1 = nc.dram_tensor("x1", [P, KCH, BC], BF, kind="ExternalInput")
    x2 = nc.dram_tensor("x2", [P, KCH, BC], BF, kind="ExternalInput")
    xd = nc.dram_tensor("xd", [P, KCH, BC], BF, kind="ExternalInput")
    h1 = nc.dram_tensor("h1", [P, KCH, BC], BF, kind="ExternalInput")
    h2 = nc.dram_tensor("h2", [P, KCH, BC], BF, kind="ExternalInput")
    hd = nc.dram_tensor("hd", [P, KCH, BC], BF, kind="ExternalInput")
    # K-major h_tm1, own columns (8 re + 8 im chunks), for rh = r*h
    hTo = nc.dram_tensor("hTo", [P, KCH, BC], BF, kind="ExternalInput")
    # batch-major h_tm1, own columns only: [256, 2048] fp32 (final combine)
    hbm = nc.dram_tensor("hbm", [BC, 2 * UC], F32, kind="ExternalInput")
    # weights, parity-sliced on host: [2048, 3*1024] (gate z|r|h col blocks,
    # z and r blocks pre-scaled by 0.2)
    RK = nc.dram_tensor("RK", [U, 3 * UC], BF, kind="ExternalInput")
    IK = nc.dram_tensor("IK", [U, 3 * UC], BF, kind="ExternalInput")
    RR = nc.dram_tensor("RR", [U, 3 * UC], BF, kind="ExternalInput")
    IR = nc.dram_tensor("IR", [U, 3 * UC], BF, kind="ExternalInput")
    # biases, own columns [re|im]: z pre-folded (0.2 b + 0.5), h plain
    biasz = nc.dram_tensor("biasz", [2 * UC], BF, kind="ExternalInput")
    biash = nc.dram_tensor("biash", [2 * UC], BF, kind="ExternalInput")
    # gate-r bias, column-major pre-folded: (0.2 b + 0.5), [128, 16]
    biasr = nc.dram_tensor("biasr", [P, KCH], F32, kind="ExternalInput")
    out = nc.dram_tensor("out", [BC, 2 * UC], F32, kind="ExternalOutput")

    hbm_r = hbm.rearrange("(m p) c -> p m c", p=P)
    out_r = out.rearrange("(m p) c -> p m c", p=P)

    with tile.TileContext(nc) as tc:
        with (
            tc.tile_pool(name="acts", bufs=1) as acts,
            tc.tile_pool(name="wpool", bufs=3) as wpool,
            tc.tile_pool(name="psum", bufs=1, space="PSUM") as psum,
            tc.tile_pool(name="small", bufs=2) as small,
            tc.tile_pool(name="bigs", bufs=1) as bigs,
            tc.tile_pool(name="dram", bufs=1, space="DRAM") as dram,
        ):
            x1s = acts.tile([P, KCH, BC], BF, tag="x1s", name="x1s")
            x2s = acts.tile([P, KCH, BC], BF, tag="x2s", name="x2s")
            xds = acts.tile([P, KCH, BC], BF, tag="xds", name="xds")
            h1s = acts.tile([P, KCH, BC], BF, tag="h1s", name="h1s")
            h2s = acts.tile([P, KCH, BC], BF, tag="h2s", name="h2s")
            hds = acts.tile([P, KCH, BC], BF, tag="hds", name="hds")
            hTos = acts.tile([P, KCH, BC], BF, tag="hTos", name="hTos")
            rh2s = acts.tile([P, KCH, BC], BF, tag="rh2s", name="rh2s")
            rhds = acts.tile([P, KCH, BC], BF, tag="rhds", name="rhds")
            # chunked act loads: first k-slices land early so the gate-r
            # matmuls start without waiting for the full set
            for o in range(0, KCH, 4):
                sl = slice(o, o + 4)
                nc.sync.dma_start(x1s[:, sl, :], x1[:, sl, :])
                nc.sync.dma_start(x2s[:, sl, :], x2[:, sl, :])
                nc.sync.dma_start(xds[:, sl, :], xd[:, sl, :])
                nc.sync.dma_start(h1s[:, sl, :], h1[:, sl, :])
                nc.sync.dma_start(h2s[:, sl, :], h2[:, sl, :])
                nc.sync.dma_start(hds[:, sl, :], hd[:, sl, :])
                nc.sync.dma_start(hTos[:, sl, :], hTo[:, sl, :])

            hbmt = bigs.tile([P, MSUB, 2 * UC], F32, tag="hbmt", name="hbmt")
            nc.sync.dma_start(hbmt[:], hbm_r)

            z_sb = bigs.tile([P, MSUB, 2 * UC], BF, tag="z_sb", name="z_sb")
            hh_sb = bigs.tile([P, MSUB, 2 * UC], BF, tag="hh_sb", name="hh_sb")

            btz = bigs.tile([P, 2 * UC], BF, tag="btz", name="btz")
            nc.sync.dma_start(btz[:], biasz[None, :].to_broadcast((P, 2 * UC)))
            bth = bigs.tile([P, 2 * UC], BF, tag="bth", name="bth")
            nc.sync.dma_start(bth[:], biash[None, :].to_broadcast((P, 2 * UC)))
            brc = small.tile([P, KCH], F32, tag="brc", name="brc", bufs=1)
            nc.sync.dma_start(brc[:], biasr[:])

            nbank = [0]

            def psum_tiles(n, shape, name):
                ts = []
                for i in range(n):
                    ts.append(
                        psum.tile(shape, F32, tag=f"ps{(nbank[0] + i) % 8}",
                                  name=f"{name}_{i}")
                    )
                nbank[0] += n
                return ts

            def wload(rep, phase, k, c0, width):
                """Stream Wr/Wi (K and R side) tiles and derive S = Wr+Wi."""
                ts = {}
                ksl = slice(k * P, (k + 1) * P)
                csl = slice(c0, c0 + width)
                for nm, src in (("wr", RK), ("wi", IK), ("wq", RR), ("wj", IR)):
                    t = wpool.tile([P, BLK], BF, tag=nm,
                                   name=f"{nm}_{phase}_{rep}_{k}")
                    nc.sync.dma_start(t[:, :width], src[ksl, csl])
                    ts[nm] = t
                sk = wpool.tile([P, BLK], BF, tag="sk",
                                name=f"sk_{phase}_{rep}_{k}")
                nc.vector.tensor_add(sk[:, :width], ts["wr"][:, :width],
                                     ts["wi"][:, :width])
                sr = wpool.tile([P, BLK], BF, tag="sr",
                                name=f"sr_{phase}_{rep}_{k}")
                nc.vector.tensor_add(sr[:, :width], ts["wq"][:, :width],
                                     ts["wj"][:, :width])
                return ts["wr"], ts["wi"], ts["wq"], ts["wj"], sk, sr

            def gate_bs(rep, g, hsrc, evict):
                """Batch-stationary gate (z: g=0, h: g=2). hsrc = the three
                recurrent-side K-major inputs (s1, s2, sdiff)."""
                s1, s2, sd = hsrc
                for bb in range(NBLK):
                    pA = psum_tiles(2, [P, BLK], f"pA_{rep}_{g}_{bb}")
                    pB = psum_tiles(2, [P, BLK], f"pB_{rep}_{g}_{bb}")
                    pC = psum_tiles(2, [P, BLK], f"pC_{rep}_{g}_{bb}")
                    for k in range(KCH):
                        wr, wi, wq, wj, sk, sr = wload(
                            rep, f"g{g}b{bb}", k, g * UC + bb * BLK, BLK
                        )
                        st = k == 0
                        sp = k == KCH - 1
                        for m in range(MSUB):
                            msl = slice(m * P, (m + 1) * P)
                            nc.tensor.matmul(pA[m], x1s[:, k, msl], wr[:],
                                             start=st, stop=False)
                            nc.tensor.matmul(pB[m], x2s[:, k, msl], wi[:],
                                             start=st, stop=False)
                            nc.tensor.matmul(pC[m], xds[:, k, msl], sk[:],
                                             start=st, stop=False)
                            nc.tensor.matmul(pA[m], s1[:, k, msl], wq[:],
                                             start=False, stop=sp)
                            nc.tensor.matmul(pB[m], s2[:, k, msl], wj[:],
                                             start=False, stop=sp)
                            nc.tensor.matmul(pC[m], sd[:, k, msl], sr[:],
                                             start=False, stop=sp)
                    for m in range(MSUB):
                        evict(rep, m, bb, pA[m], pB[m], pC[m])

            def evict_z(rep, m, bb, A, B, C):
                for half in range(2):
                    osl = slice(half * UC + bb * BLK,
                                half * UC + (bb + 1) * BLK)
                    t = small.tile([P, BLK], F32, tag="tmp",
                                   name=f"tz_{rep}_{m}_{bb}_{half}")
                    # one PSUM operand per DVE op: bias first, then B (, C)
                    nc.vector.tensor_add(t[:], A[:], btz[:, osl])
                    if half == 0:
                        nc.vector.tensor_add(t[:], t[:], B[:])
                    else:
                        nc.vector.tensor_sub(t[:], t[:], B[:])
                        nc.vector.tensor_add(t[:], t[:], C[:])
                    nc.vector.tensor_scalar(
                        z_sb[:, m, osl], t[:], 1.0, 0.0, AL.min, AL.max)

            def evict_h(rep, m, bb, A, B, C):
                for half in range(2):
                    osl = slice(half * UC + bb * BLK,
                                half * UC + (bb + 1) * BLK)
                    t = small.tile([P, BLK], F32, tag="tmp",
                                   name=f"th_{rep}_{m}_{bb}_{half}")
                    nc.vector.tensor_add(t[:], A[:], bth[:, osl])
                    if half == 0:
                        nc.vector.tensor_add(t[:], t[:], B[:])
                    else:
                        nc.vector.tensor_sub(t[:], t[:], B[:])
                        nc.vector.tensor_add(t[:], t[:], C[:])
                    nc.scalar.activation(
                        hh_sb[:, m, osl], t[:],
                        mybir.ActivationFunctionType.Tanh)

            def gate_r(rep, rhTl):
                """Gate r (g=1), output-transposed: for each cc-pair pass,
                6 psums of [128 wcols, 256 batch]. Writes rh = hs(pre)*h
                directly into rhTl K-major chunks."""
                for grp in range(2):
                    for cp in range(2):
                        c0 = UC + grp * BLK + cp * 256
                        pA = psum_tiles(2, [P, BC], f"rA_{rep}_{grp}_{cp}")
                        pB = psum_tiles(2, [P, BC], f"rB_{rep}_{grp}_{cp}")
                        pC = psum_tiles(2, [P, BC], f"rC_{rep}_{grp}_{cp}")
                        for k in range(KCH):
                            wr, wi, wq, wj, sk, sr = wload(
                                rep, f"r{grp}{cp}", k, c0, 256
                            )
                            st = k == 0
                            sp = k == KCH - 1
                            for cc in range(2):
                                csl = slice(cc * P, (cc + 1) * P)
                                nc.tensor.matmul(pA[cc], wr[:, csl],
                                                 x1s[:, k, :],
                                                 start=st, stop=False)
                                nc.tensor.matmul(pB[cc], wi[:, csl],
                                                 x2s[:, k, :],
                                                 start=st, stop=False)
                                nc.tensor.matmul(pC[cc], sk[:, csl],
                                                 xds[:, k, :],
                                                 start=st, stop=False)
                                nc.tensor.matmul(pA[cc], wq[:, csl],
                                                 h1s[:, k, :],
                                                 start=False, stop=sp)
                                nc.tensor.matmul(pB[cc], wj[:, csl],
                                                 h2s[:, k, :],
                                                 start=False, stop=sp)
                                nc.tensor.matmul(pC[cc], sr[:, csl],
                                                 hds[:, k, :],
                                                 start=False, stop=sp)
                        for cc in range(2):
                            ccg = grp * 4 + cp * 2 + cc
                            t = small.tile([P, BC], F32, tag="rtmp",
                                           name=f"tr_{rep}_{ccg}")
                            # real: clip((A + br) + B) * h_re
                            nc.vector.tensor_scalar(
                                t[:], pA[cc][:], 1.0, brc[:, ccg:ccg + 1],
                                AL.mult, AL.add)
                            nc.vector.tensor_add(t[:], t[:], pB[cc][:])
                            nc.vector.tensor_scalar(
                                t[:], t[:], 1.0, 0.0, AL.min, AL.max)
                            nc.vector.tensor_mul(
                                rhTl[:, ccg, :], t[:], hTos[:, ccg, :])
                            # imag: clip((A + bi) - B + C) * h_im
                            t2 = small.tile([P, BC], F32, tag="rtmp",
                                            name=f"ti_{rep}_{ccg}")
                            nc.vector.tensor_scalar(
                                t2[:], pA[cc][:], 1.0,
                                brc[:, 8 + ccg:9 + ccg], AL.mult, AL.add)
                            nc.vector.tensor_sub(t2[:], t2[:], pB[cc][:])
                            nc.vector.tensor_add(t2[:], t2[:], pC[cc][:])
                            nc.vector.tensor_scalar(
                                t2[:], t2[:], 1.0, 0.0, AL.min, AL.max)
                            nc.vector.tensor_mul(
                                rhTl[:, 8 + ccg, :], t2[:],
                                hTos[:, 8 + ccg, :])

            for rep in range(repeat):
                # --- gate r first (g=1), output-transposed ---
                rhTl = acts.tile([P, KCH, BC], BF, tag="rh1s",
                                 name=f"rhTl_{rep}")
                gate_r(rep, rhTl)

                # pairwise AllGather of rhT
                inb = dram.tile([P, KCH, BC], BF, tag="inb",
                                name=f"inb_{rep}")
                outb = dram.tile([2, P, KCH, BC], BF, tag="outb",
                                 name=f"outb_{rep}")
                nc.sync.dma_start(inb[:], rhTl[:])
                nc.gpsimd.collective_compute(
                    "AllGather",
                    mybir.AluOpType.bypass,
                    replica_groups=[[0, 1], [2, 3], [4, 5], [6, 7]],
                    ins=[inb[:].opt()],
                    outs=[outb[:].opt()],
                )

                # --- gate z (g=0), overlaps with the collective ---
                gate_bs(rep, 0, (h1s, h2s, hds), evict_z)

                # unpack: real chunks of both parities -> rh1s, imag -> rh2s
                rh1s = acts.tile([P, KCH, BC], BF, tag="rh1s",
                                 name=f"rh1s_{rep}")
                nc.sync.dma_start(rh1s[:, 0:8, :], outb[0, :, 0:8, :])
                nc.sync.dma_start(rh1s[:, 8:16, :], outb[1, :, 0:8, :])
                nc.sync.dma_start(rh2s[:, 0:8, :], outb[0, :, 8:16, :])
                nc.sync.dma_start(rh2s[:, 8:16, :], outb[1, :, 8:16, :])
                for o in range(0, KCH, 4):
                    sl = slice(o, o + 4)
                    nc.vector.tensor_sub(rhds[:, sl, :], rh2s[:, sl, :],
                                         rh1s[:, sl, :])

                # --- gate h (g=2) ---
                gate_bs(rep, 2, (rh1s, rh2s, rhds), evict_h)

                # h_new = hh + z*(h - hh), in place into hbmt;
                # per m-subtile so DVE of one overlaps the out-DMA of the other
                for m in range(MSUB):
                    nc.vector.tensor_sub(
                        hbmt[:, m, :], hbmt[:, m, :], hh_sb[:, m, :]
                    )
                    nc.vector.tensor_mul(
                        hbmt[:, m, :], z_sb[:, m, :], hbmt[:, m, :]
                    )
                    nc.vector.tensor_add(
                        hbmt[:, m, :], hh_sb[:, m, :], hbmt[:, m, :]
                    )
                    nc.sync.dma_start(out_r[:, m, :], hbmt[:, m, :])

    nc.compile()
    return nc


def _pack_kmajor(a):
    # (BC, 2048) -> (128, 16, BC) with [p, o, b] = a[b, o*128+p]
    bc = a.shape[0]
    return np.ascontiguousarray(a.T.reshape(KCH, P, bc).transpose(1, 0, 2))


def _bf(a):
    return np.asarray(a, dtype=np.float32).astype(NPBF)


def make_in_maps(
    inputs, h_tm1, real_kernel, imaginary_kernel,
    real_recurrent_kernel, imaginary_recurrent_kernel, real_bias,
    imaginary_bias,
):
    inputs = np.ascontiguousarray(inputs, dtype=np.float32)
    h_tm1 = np.ascontiguousarray(h_tm1, dtype=np.float32)
    ws = {
        "RK": np.asarray(real_kernel, dtype=np.float32),
        "IK": np.asarray(imaginary_kernel, dtype=np.float32),
        "RR": np.asarray(real_recurrent_kernel, dtype=np.float32),
        "IR": np.asarray(imaginary_recurrent_kernel, dtype=np.float32),
    }
    rb = np.asarray(real_bias, dtype=np.float32)
    ib = np.asarray(imaginary_bias, dtype=np.float32)

    wsl = {}
    bz = {}
    bh = {}
    brc = {}
    for p in range(2):
        cols = [slice(g * U + p * UC, g * U + (p + 1) * UC) for g in range(3)]
        wsl[p] = {}
        for k, v in ws.items():
            w = np.concatenate([v[:, c] for c in cols], axis=1).copy()
            w[:, : 2 * UC] *= 0.2  # z and r gate blocks pre-scaled
            wsl[p][k] = np.ascontiguousarray(_bf(w))
        bz[p] = _bf(0.2 * np.concatenate([rb[cols[0]], ib[cols[0]]]) + 0.5)
        bh[p] = _bf(np.concatenate([rb[cols[2]], ib[cols[2]]]))
        br = np.concatenate([rb[cols[1]], ib[cols[1]]])
        brc[p] = np.ascontiguousarray(
            (0.2 * br + 0.5).reshape(KCH, P).T.astype(np.float32)
        )

    x1f = inputs[:, :U]
    x2f = inputs[:, U:]
    xdf = x2f - x1f
    h1f = h_tm1[:, :U]
    h2f = h_tm1[:, U:]
    hdf = h2f - h1f

    in_maps = []
    for c in range(N_CORES):
        g, p = c // 2, c % 2
        rs = slice(g * BC, (g + 1) * BC)
        ocr = slice(p * UC, (p + 1) * UC)
        oci = slice(U + p * UC, U + (p + 1) * UC)
        hbm = np.ascontiguousarray(
            np.concatenate([h_tm1[rs, ocr], h_tm1[rs, oci]], axis=1)
        )
        in_maps.append(
            {
                "x1": _pack_kmajor(_bf(x1f[rs])),
                "x2": _pack_kmajor(_bf(x2f[rs])),
                "xd": _pack_kmajor(_bf(xdf[rs])),
                "h1": _pack_kmajor(_bf(h1f[rs])),
                "h2": _pack_kmajor(_bf(h2f[rs])),
                "hd": _pack_kmajor(_bf(hdf[rs])),
                "hTo": _pack_kmajor(_bf(hbm)),
                "hbm": hbm,
                "RK": wsl[p]["RK"],
                "IK": wsl[p]["IK"],
                "RR": wsl[p]["RR"],
                "IR": wsl[p]["IR"],
                "biasz": bz[p],
                "biash": bh[p],
                "biasr": brc[p],
            }
        )
    return in_maps


def scatter_out(results):
    h_new = np.empty((B, 2 * U), dtype=np.float32)
    for c in range(N_CORES):
        g, p = c // 2, c % 2
        rs = slice(g * BC, (g + 1) * BC)
        o = results[c]["out"]
        h_new[rs, p * UC : (p + 1) * UC] = o[:, :UC]
        h_new[rs, U + p * UC : U + (p + 1) * UC] = o[:, UC:]
    return h_new


def _build_nc_retry(repeat=1, attempts=4):
    # Tile's scheduler very occasionally reports a spurious deadlock on a
    # valid graph (ordering is not fully deterministic); retry a few times.
    last = None
    for _ in range(attempts):
        try:
            return _build_nc(repeat=repeat)
        except Exception as e:  # noqa: BLE001
            if "Deadlock" not in type(e).__name__ + str(e):
                raise
            last = e
    raise last


def kernel(
    inputs,
    h_tm1,
    real_kernel,
    imaginary_kernel,
    real_recurrent_kernel,
    imaginary_recurrent_kernel,
    real_bias,
    imaginary_bias,
):
    if "nc" not in _CACHE:
        _CACHE["nc"] = _build_nc_retry()
    nc = _CACHE["nc"]
    in_maps = make_in_maps(
        inputs, h_tm1, real_kernel, imaginary_kernel,
        real_recurrent_kernel, imaginary_recurrent_kernel, real_bias,
        imaginary_bias,
    )
    res = run_bass_kernel_spmd(nc, in_maps, core_ids=list(range(N_CORES)))
    return scatter_out(res.results)


# revision 18
# speedup vs baseline: 1.5591x; 1.0196x over previous
"""CGRUCell Trainium2 kernel — hybrid DP4 x TP2 on 8 NeuronCores, with a
3-product complex-matmul decomposition in bf16.

Core c: batch group g=c//2 (256 rows), unit parity p=c%2 (units
[p*1024:(p+1)*1024]). The complex matmul [x1 x2] @ [[Wr,-Wi],[Wi,Wr]]
is computed with 3 half-size products instead of 4 (Gauss/Karatsuba):

    A = x1 @ Wr,  B = x2 @ Wi,  C = (x2 - x1) @ (Wr + Wi)
    real = A + B,   imag = C + A - B

This cuts tensor-engine cycles by 25% vs the 4-product form (the PE
streams bf16 and fp32r at the same 1 column/cycle, so dtype alone does
not help compute; fp8-DoubleRow would, but e4m3 fails the accuracy
budget). Weights stream from HBM in bf16 (half the bytes of the fp32r
baseline); only the combination S = Wr + Wi is derived on-chip by the
vector engine, and the activation differences (x2-x1 etc.) come
pre-computed from the host, so HBM traffic stays ~65 MB/core, well
under the ~358 GB/s/core limit at the 246 us PE roofline.

z- and r-gate weight blocks are pre-scaled by 0.2 on the host so the
hard-sigmoid scale comes free; biases fold into fused
scalar_tensor_tensor / tensor_scalar evictions.

Gate r is computed output-transposed (weights stationary) so r*h is
produced directly in K-major layout for the pairwise AllGather, which
overlaps with gate z. Gates z and h are batch-stationary; per 512-col
block the three Gauss accumulators for both 128-row batch subtiles
occupy 6 PSUM banks, rotating through all 8 banks across passes.
"""

import sys

for _p in ("/opt/trn_rl_repo", "/root/.axon_site/_ro/trn_rl_repo"):
    if _p not in sys.path:
        sys.path.append(_p)

import numpy as np
import ml_dtypes

import concourse.bass as bass
import concourse.mybir as mybir
import concourse.tile as tile
from concourse import bacc
from concourse.bass_utils import run_bass_kernel_spmd

P = 128
U = 2048           # UNITS
B = 1024
N_CORES = 8
GROUPS = 4
BC = B // GROUPS   # 256 batch rows per core
MSUB = BC // P     # 2 m-subtiles
UC = U // 2        # 1024 unit columns per core (per half)
KCH = U // P       # 16 k-chunks per complex half
F32 = mybir.dt.float32
BF = mybir.dt.bfloat16
NPBF = ml_dtypes.bfloat16
NBLK = UC // 512   # 2 col-blocks of 512 per half
BLK = 512

AL = mybir.AluOpType

_CACHE = {}


def _build_nc(repeat=1):
    nc = bacc.Bacc(None, target_bir_lowering=False)

    # K-major activations: x halves, their difference, h halves + diff
    x1 = nc.dram_tensor("x1", [P, KCH, BC], BF, kind="ExternalInput")
    x2 = nc.dram_tensor("x2", [P, KCH, BC], BF, kind="ExternalInput")
    xd = nc.dram_tensor("xd", [P, KCH, BC], BF, kind="ExternalInput")
    h1 = nc.dram_tensor("h1", [P, KCH, BC], BF, kind="ExternalInput")
    h2 = nc.dram_tensor("h2", [P, KCH, BC], BF, kind="ExternalInput")
    hd = nc.dram_tensor("hd", [P, KCH, BC], BF, kind="ExternalInput")
    # K-major h_tm1, own columns (8 re + 8 im chunks), for rh = r*h
    hTo = nc.dram_tensor("hTo", [P, KCH, BC], BF, kind="ExternalInput")
    # batch-major h_tm1, own columns only: [256, 2048] fp32 (final combine)
    hbm = nc.dram_tensor("hbm", [BC, 2 * UC], F32, kind="ExternalInput")
    # weights, parity-sliced and repacked on host so one DMA per (k-chunk,
    # 512-col block) brings all four matrices as contiguous 4KB rows:
    # W4[k, blk, p, j, c] = Wj[k*128+p, blk*512+c], j in (RK, IK, RR, IR);
    # z and r gate blocks pre-scaled by 0.2
    W4 = nc.dram_tensor("W4", [KCH, 6, P, 4, BLK], BF, kind="ExternalInput")
    # biases, own columns [re|im]: z pre-folded (0.2 b + 0.5), h plain
    biasz = nc.dram_tensor("biasz", [2 * UC], BF, kind="ExternalInput")
    biash = nc.dram_tensor("biash", [2 * UC], BF, kind="ExternalInput")
    # gate-r bias, column-major pre-folded: (0.2 b + 0.5), [128, 16]
    biasr = nc.dram_tensor("biasr", [P, KCH], F32, kind="ExternalInput")
    out = nc.dram_tensor("out", [BC, 2 * UC], F32, kind="ExternalOutput")

    hbm_r = hbm.rearrange("(m p) c -> p m c", p=P)
    out_r = out.rearrange("(m p) c -> p m c", p=P)

    with tile.TileContext(nc) as tc:
        with (
            tc.tile_pool(name="acts", bufs=1) as acts,
            tc.tile_pool(name="wpool", bufs=3) as wpool,
            tc.tile_pool(name="psum", bufs=1, space="PSUM") as psum,
            tc.tile_pool(name="small", bufs=2) as small,
            tc.tile_pool(name="bigs", bufs=1) as bigs,
            tc.tile_pool(name="dram", bufs=1, space="DRAM") as dram,
        ):
            x1s = acts.tile([P, KCH, BC], BF, tag="x1s", name="x1s")
            x2s = acts.tile([P, KCH, BC], BF, tag="x2s", name="x2s")
            xds = acts.tile([P, KCH, BC], BF, tag="xds", name="xds")
            h1s = acts.tile([P, KCH, BC], BF, tag="h1s", name="h1s")
            h2s = acts.tile([P, KCH, BC], BF, tag="h2s", name="h2s")
            hds = acts.tile([P, KCH, BC], BF, tag="hds", name="hds")
            hTos = acts.tile([P, KCH, BC], BF, tag="hTos", name="hTos")
            rh2s = acts.tile([P, KCH, BC], BF, tag="rh2s", name="rh2s")
            rhds = acts.tile([P, KCH, BC], BF, tag="rhds", name="rhds")
            # chunked act loads: a tiny first slice so the gate-r k=0
            # matmuls start ASAP, then coarser chunks
            for lo, hi in ((0, 1), (1, 4), (4, 8), (8, 12), (12, 16)):
                sl = slice(lo, hi)
                nc.sync.dma_start(x1s[:, sl, :], x1[:, sl, :])
                nc.sync.dma_start(x2s[:, sl, :], x2[:, sl, :])
                nc.sync.dma_start(xds[:, sl, :], xd[:, sl, :])
                nc.sync.dma_start(h1s[:, sl, :], h1[:, sl, :])
                nc.sync.dma_start(h2s[:, sl, :], h2[:, sl, :])
                nc.sync.dma_start(hds[:, sl, :], hd[:, sl, :])
                nc.sync.dma_start(hTos[:, sl, :], hTo[:, sl, :])

            hbmt = bigs.tile([P, MSUB, 2 * UC], F32, tag="hbmt", name="hbmt")
            nc.sync.dma_start(hbmt[:], hbm_r)

            z_sb = bigs.tile([P, MSUB, 2 * UC], BF, tag="z_sb", name="z_sb")
            hh_sb = bigs.tile([P, MSUB, 2 * UC], BF, tag="hh_sb", name="hh_sb")

            btz = bigs.tile([P, 2 * UC], BF, tag="btz", name="btz")
            nc.sync.dma_start(btz[:], biasz[None, :].to_broadcast((P, 2 * UC)))
            bth = bigs.tile([P, 2 * UC], BF, tag="bth", name="bth")
            nc.sync.dma_start(bth[:], biash[None, :].to_broadcast((P, 2 * UC)))
            brc = small.tile([P, KCH], F32, tag="brc", name="brc", bufs=1)
            nc.sync.dma_start(brc[:], biasr[:])

            nbank = [0]

            def psum_tiles(n, shape, name):
                ts = []
                for i in range(n):
                    ts.append(
                        psum.tile(shape, F32, tag=f"ps{(nbank[0] + i) % 8}",
                                  name=f"{name}_{i}")
                    )
                nbank[0] += n
                return ts

            def wload(rep, phase, k, blk, cp0, width):
                """Stream all four weight tiles for (k, 512-col block) in one
                DMA and derive S = Wr+Wi (K and R side) on the DVE."""
                wt = wpool.tile([P, 4, width], BF, tag="wt",
                                name=f"wt_{phase}_{rep}_{k}")
                nc.sync.dma_start(
                    wt[:], W4[k, blk, :, :, cp0:cp0 + width])
                sk = wpool.tile([P, BLK], BF, tag="sk",
                                name=f"sk_{phase}_{rep}_{k}")
                nc.vector.tensor_add(sk[:, :width], wt[:, 0, :], wt[:, 1, :])
                sr = wpool.tile([P, BLK], BF, tag="sr",
                                name=f"sr_{phase}_{rep}_{k}")
                nc.vector.tensor_add(sr[:, :width], wt[:, 2, :], wt[:, 3, :])
                return wt, sk, sr

            def gate_bs(rep, g, hsrc, evict):
                """Batch-stationary gate (z: g=0, h: g=2). hsrc = the three
                recurrent-side K-major inputs (s1, s2, sdiff)."""
                s1, s2, sd = hsrc
                for bb in range(NBLK):
                    pA = psum_tiles(2, [P, BLK], f"pA_{rep}_{g}_{bb}")
                    pB = psum_tiles(2, [P, BLK], f"pB_{rep}_{g}_{bb}")
                    pC = psum_tiles(2, [P, BLK], f"pC_{rep}_{g}_{bb}")
                    for k in range(KCH):
                        wt, sk, sr = wload(
                            rep, f"g{g}b{bb}", k, g * 2 + bb, 0, BLK
                        )
                        st = k == 0
                        sp = k == KCH - 1
                        for m in range(MSUB):
                            msl = slice(m * P, (m + 1) * P)
                            nc.tensor.matmul(pA[m], x1s[:, k, msl],
                                             wt[:, 0, :],
                                             start=st, stop=False)
                            nc.tensor.matmul(pB[m], x2s[:, k, msl],
                                             wt[:, 1, :],
                                             start=st, stop=False)
                            nc.tensor.matmul(pC[m], xds[:, k, msl], sk[:],
                                             start=st, stop=False)
                            nc.tensor.matmul(pA[m], s1[:, k, msl],
                                             wt[:, 2, :],
                                             start=False, stop=sp)
                            nc.tensor.matmul(pB[m], s2[:, k, msl],
                                             wt[:, 3, :],
                                             start=False, stop=sp)
                            nc.tensor.matmul(pC[m], sd[:, k, msl], sr[:],
                                             start=False, stop=sp)
                    for m in range(MSUB):
                        evict(rep, m, bb, pA[m], pB[m], pC[m])

            def evict_z(rep, m, bb, A, B, C):
                for half in range(2):
                    osl = slice(half * UC + bb * BLK,
                                half * UC + (bb + 1) * BLK)
                    t = small.tile([P, BLK], F32, tag="tmp",
                                   name=f"tz_{rep}_{m}_{bb}_{half}")
                    # one PSUM operand per DVE op: bias first, then B (, C)
                    nc.vector.tensor_add(t[:], A[:], btz[:, osl])
                    if half == 0:
                        nc.vector.tensor_add(t[:], t[:], B[:])
                    else:
                        nc.vector.tensor_sub(t[:], t[:], B[:])
                        nc.vector.tensor_add(t[:], t[:], C[:])
                    nc.vector.tensor_scalar(
                        z_sb[:, m, osl], t[:], 1.0, 0.0, AL.min, AL.max)

            def evict_h(rep, m, bb, A, B, C):
                for half in range(2):
                    osl = slice(half * UC + bb * BLK,
                                half * UC + (bb + 1) * BLK)
                    t = small.tile([P, BLK], F32, tag="tmp",
                                   name=f"th_{rep}_{m}_{bb}_{half}")
                    nc.vector.tensor_add(t[:], A[:], bth[:, osl])
                    if half == 0:
                        nc.vector.tensor_add(t[:], t[:], B[:])
                    else:
                        nc.vector.tensor_sub(t[:], t[:], B[:])
                        nc.vector.tensor_add(t[:], t[:], C[:])
                    nc.scalar.activation(
                        hh_sb[:, m, osl], t[:],
                        mybir.ActivationFunctionType.Tanh)
                    # fused final combine + store for this tile, so the
                    # h_new tail streams behind the evictions instead of
                    # serializing after the last matmul:
                    # h = hh + z*(h - hh), in place into hbmt
                    nc.vector.tensor_sub(
                        hbmt[:, m, osl], hbmt[:, m, osl], hh_sb[:, m, osl])
                    nc.vector.tensor_mul(
                        hbmt[:, m, osl], z_sb[:, m, osl], hbmt[:, m, osl])
                    nc.vector.tensor_add(
                        hbmt[:, m, osl], hh_sb[:, m, osl], hbmt[:, m, osl])
                    nc.sync.dma_start(out_r[:, m, osl], hbmt[:, m, osl])

            def gate_r(rep, rhTl):
                """Gate r (g=1), output-transposed: for each cc-pair pass,
                6 psums of [128 wcols, 256 batch]. Writes rh = hs(pre)*h
                directly into rhTl K-major chunks."""
                for grp in range(2):
                    for cp in range(2):
                        pA = psum_tiles(2, [P, BC], f"rA_{rep}_{grp}_{cp}")
                        pB = psum_tiles(2, [P, BC], f"rB_{rep}_{grp}_{cp}")
                        pC = psum_tiles(2, [P, BC], f"rC_{rep}_{grp}_{cp}")
                        for k in range(KCH):
                            wt, sk, sr = wload(
                                rep, f"r{grp}{cp}", k, 2 + grp, cp * 256, 256
                            )
                            st = k == 0
                            sp = k == KCH - 1
                            for cc in range(2):
                                csl = slice(cc * P, (cc + 1) * P)
                                nc.tensor.matmul(pA[cc], wt[:, 0, csl],
                                                 x1s[:, k, :],
                                                 start=st, stop=False)
                                nc.tensor.matmul(pB[cc], wt[:, 1, csl],
                                                 x2s[:, k, :],
                                                 start=st, stop=False)
                                nc.tensor.matmul(pC[cc], sk[:, csl],
                                                 xds[:, k, :],
                                                 start=st, stop=False)
                                nc.tensor.matmul(pA[cc], wt[:, 2, csl],
                                                 h1s[:, k, :],
                                                 start=False, stop=sp)
                                nc.tensor.matmul(pB[cc], wt[:, 3, csl],
                                                 h2s[:, k, :],
                                                 start=False, stop=sp)
                                nc.tensor.matmul(pC[cc], sr[:, csl],
                                                 hds[:, k, :],
                                                 start=False, stop=sp)
                        for cc in range(2):
                            ccg = grp * 4 + cp * 2 + cc
                            t = small.tile([P, BC], F32, tag="rtmp",
                                           name=f"tr_{rep}_{ccg}")
                            # real: clip((A + br) + B) * h_re
                            nc.vector.tensor_scalar(
                                t[:], pA[cc][:], 1.0, brc[:, ccg:ccg + 1],
                                AL.mult, AL.add)
                            nc.vector.tensor_add(t[:], t[:], pB[cc][:])
                            nc.vector.tensor_scalar(
                                t[:], t[:], 1.0, 0.0, AL.min, AL.max)
                            nc.vector.tensor_mul(
                                rhTl[:, ccg, :], t[:], hTos[:, ccg, :])
                            # imag: clip((A + bi) - B + C) * h_im
                            t2 = small.tile([P, BC], F32, tag="rtmp",
                                            name=f"ti_{rep}_{ccg}")
                            nc.vector.tensor_scalar(
                                t2[:], pA[cc][:], 1.0,
                                brc[:, 8 + ccg:9 + ccg], AL.mult, AL.add)
                            nc.vector.tensor_sub(t2[:], t2[:], pB[cc][:])
                            nc.vector.tensor_add(t2[:], t2[:], pC[cc][:])
                            nc.vector.tensor_scalar(
                                t2[:], t2[:], 1.0, 0.0, AL.min, AL.max)
                            nc.vector.tensor_mul(
                                rhTl[:, 8 + ccg, :], t2[:],
                                hTos[:, 8 + ccg, :])

            for rep in range(repeat):
                # --- gate r first (g=1), output-transposed ---
                rhTl = acts.tile([P, KCH, BC], BF, tag="rh1s",
                                 name=f"rhTl_{rep}")
                gate_r(rep, rhTl)

                # pairwise AllGather of rhT
                inb = dram.tile([P, KCH, BC], BF, tag="inb",
                                name=f"inb_{rep}")
                outb = dram.tile([2, P, KCH, BC], BF, tag="outb",
                                 name=f"outb_{rep}")
                nc.sync.dma_start(inb[:], rhTl[:])
                nc.gpsimd.collective_compute(
                    "AllGather",
                    mybir.AluOpType.bypass,
                    replica_groups=[[0, 1], [2, 3], [4, 5], [6, 7]],
                    ins=[inb[:].opt()],
                    outs=[outb[:].opt()],
                )

                # --- gate z (g=0), overlaps with the collective ---
                gate_bs(rep, 0, (h1s, h2s, hds), evict_z)

                # unpack: real chunks of both parities -> rh1s, imag -> rh2s
                rh1s = acts.tile([P, KCH, BC], BF, tag="rh1s",
                                 name=f"rh1s_{rep}")
                nc.sync.dma_start(rh1s[:, 0:8, :], outb[0, :, 0:8, :])
                nc.sync.dma_start(rh1s[:, 8:16, :], outb[1, :, 0:8, :])
                nc.sync.dma_start(rh2s[:, 0:8, :], outb[0, :, 8:16, :])
                nc.sync.dma_start(rh2s[:, 8:16, :], outb[1, :, 8:16, :])
                for o in range(0, KCH, 4):
                    sl = slice(o, o + 4)
                    nc.vector.tensor_sub(rhds[:, sl, :], rh2s[:, sl, :],
                                         rh1s[:, sl, :])

                # --- gate h (g=2); evict_h fuses the final combine+store ---
                gate_bs(rep, 2, (rh1s, rh2s, rhds), evict_h)

    nc.compile()
    return nc


def _pack_kmajor(a):
    # (BC, 2048) -> (128, 16, BC) with [p, o, b] = a[b, o*128+p]
    bc = a.shape[0]
    return np.ascontiguousarray(a.T.reshape(KCH, P, bc).transpose(1, 0, 2))


def _bf(a):
    return np.asarray(a, dtype=np.float32).astype(NPBF)


def make_in_maps(
    inputs, h_tm1, real_kernel, imaginary_kernel,
    real_recurrent_kernel, imaginary_recurrent_kernel, real_bias,
    imaginary_bias,
):
    inputs = np.ascontiguousarray(inputs, dtype=np.float32)
    h_tm1 = np.ascontiguousarray(h_tm1, dtype=np.float32)
    ws = {
        "RK": np.asarray(real_kernel, dtype=np.float32),
        "IK": np.asarray(imaginary_kernel, dtype=np.float32),
        "RR": np.asarray(real_recurrent_kernel, dtype=np.float32),
        "IR": np.asarray(imaginary_recurrent_kernel, dtype=np.float32),
    }
    rb = np.asarray(real_bias, dtype=np.float32)
    ib = np.asarray(imaginary_bias, dtype=np.float32)

    w4 = {}
    bz = {}
    bh = {}
    brc = {}
    for p in range(2):
        cols = [slice(g * U + p * UC, g * U + (p + 1) * UC) for g in range(3)]
        stk = []
        for k in ("RK", "IK", "RR", "IR"):
            w = np.concatenate([ws[k][:, c] for c in cols], axis=1).copy()
            w[:, : 2 * UC] *= 0.2  # z and r gate blocks pre-scaled
            stk.append(_bf(w))
        # [4, 2048, 3072] -> [KCH, 6 blocks, 128, 4, 512]
        w4[p] = np.ascontiguousarray(
            np.stack(stk)
            .reshape(4, KCH, P, 6, BLK)
            .transpose(1, 3, 2, 0, 4)
        )
        bz[p] = _bf(0.2 * np.concatenate([rb[cols[0]], ib[cols[0]]]) + 0.5)
        bh[p] = _bf(np.concatenate([rb[cols[2]], ib[cols[2]]]))
        br = np.concatenate([rb[cols[1]], ib[cols[1]]])
        brc[p] = np.ascontiguousarray(
            (0.2 * br + 0.5).reshape(KCH, P).T.astype(np.float32)
        )

    x1f = inputs[:, :U]
    x2f = inputs[:, U:]
    xdf = x2f - x1f
    h1f = h_tm1[:, :U]
    h2f = h_tm1[:, U:]
    hdf = h2f - h1f

    in_maps = []
    for c in range(N_CORES):
        g, p = c // 2, c % 2
        rs = slice(g * BC, (g + 1) * BC)
        ocr = slice(p * UC, (p + 1) * UC)
        oci = slice(U + p * UC, U + (p + 1) * UC)
        hbm = np.ascontiguousarray(
            np.concatenate([h_tm1[rs, ocr], h_tm1[rs, oci]], axis=1)
        )
        in_maps.append(
            {
                "x1": _pack_kmajor(_bf(x1f[rs])),
                "x2": _pack_kmajor(_bf(x2f[rs])),
                "xd": _pack_kmajor(_bf(xdf[rs])),
                "h1": _pack_kmajor(_bf(h1f[rs])),
                "h2": _pack_kmajor(_bf(h2f[rs])),
                "hd": _pack_kmajor(_bf(hdf[rs])),
                "hTo": _pack_kmajor(_bf(hbm)),
                "hbm": hbm,
                "W4": w4[p],
                "biasz": bz[p],
                "biash": bh[p],
                "biasr": brc[p],
            }
        )
    return in_maps


def scatter_out(results):
    h_new = np.empty((B, 2 * U), dtype=np.float32)
    for c in range(N_CORES):
        g, p = c // 2, c % 2
        rs = slice(g * BC, (g + 1) * BC)
        o = results[c]["out"]
        h_new[rs, p * UC : (p + 1) * UC] = o[:, :UC]
        h_new[rs, U + p * UC : U + (p + 1) * UC] = o[:, UC:]
    return h_new


def _build_nc_retry(repeat=1, attempts=4):
    # Tile's scheduler very occasionally reports a spurious deadlock on a
    # valid graph (ordering is not fully deterministic); retry a few times.
    last = None
    for _ in range(attempts):
        try:
            return _build_nc(repeat=repeat)
        except Exception as e:  # noqa: BLE001
            if "Deadlock" not in type(e).__name__ + str(e):
                raise
            last = e
    raise last


def kernel(
    inputs,
    h_tm1,
    real_kernel,
    imaginary_kernel,
    real_recurrent_kernel,
    imaginary_recurrent_kernel,
    real_bias,
    imaginary_bias,
):
    if "nc" not in _CACHE:
        _CACHE["nc"] = _build_nc_retry()
    nc = _CACHE["nc"]
    in_maps = make_in_maps(
        inputs, h_tm1, real_kernel, imaginary_kernel,
        real_recurrent_kernel, imaginary_recurrent_kernel, real_bias,
        imaginary_bias,
    )
    res = run_bass_kernel_spmd(nc, in_maps, core_ids=list(range(N_CORES)))
    return scatter_out(res.results)
